# revision 59
# baseline (speedup 1.0000x reference)
"""Trainium2 Bass kernel for nn_CSG2A_net (gnn_message_passing).

Math (identical to reference, [B,G,G] score never materialized):
  CCE:  h = relu(node_feat @ W1); w = adj*exp(-dist)
        g[b,m] = sum_n mask[b,n] w[b,n,m]
        pooled[b,d] = (sum_m g[b,m] h[b,m,d]) / clip(mask.sum, 1)
        comp = pooled @ W2 + dose @ w_dose + time @ w_time
  u = b_gex @ w_gex + comp @ w_comp                       [B,H]
  A = u @ w_gex.T/sqrt(H); C = u @ w_comp.T/sqrt(H)       [B,G]
  pred = b_gex * (b_gex*A + comp*C + ppi.sum(-1))
  out  = relu(LN(pred)) @ W_ff

Cost-model-driven design, data-parallel over batch (8 cores x 8
samples, weights replicated):
 - Host pre-packs every tensor into its exact SBUF tile layout (pure
   marshalling) and compresses dtypes: bf16 for all PE operands, fp8e4m3
   for ppi_adj (only row sums are needed; quantization averages out).
   Per-core traffic: 8.7MB -> ~3.7MB.
 - HWDGE DMA blocks its issuing engine through the transfer and
   per-engine DMAs serialize, so bytes are spread over three queues:
   SP (no compute), plus early-idle windows on ACT and Pool.
 - Weights ride the stationary side of every matmul (ppi row sums are
   64 one-column matmuls of a k-major pack; FFN moves only xn).
 - ACT activation-table loads (1.3us each) are hoisted: Exp first,
   then a dummy Sqrt switch; Copy/Relu live in every table set.
 - Outputs leave gene-major; the host transposes them back.
"""

import numpy as np
import ml_dtypes

import concourse.bass as bass
import concourse.mybir as mybir
import concourse.tile as tile
from concourse.bass_utils import run_bass_kernel_spmd

F32 = mybir.dt.float32
BF16 = mybir.dt.bfloat16
FP8 = mybir.dt.float8e4
AF = mybir.ActivationFunctionType
AX = mybir.AxisListType
OP = mybir.AluOpType

NP_BF16 = ml_dtypes.bfloat16
NP_FP8 = ml_dtypes.float8_e4m3

G, H, NA, FEAT, CH = 978, 128, 50, 34, 64
B, NCORES = 64, 8
BL = B // NCORES
NGT = 8          # gene tiles: 7 x 128 + 82 (rows 978..1023 zero-padded)
GP = 1024
LN_EPS = 1e-5
ISH = 1.0 / float(np.sqrt(H))

# in50 packed columns: adjT | distT | maskT | nfT | W1
ADJ0, DIST0, MSK0, NF0, W10 = 0, 400, 800, 808, 1208
IN50_W = W10 + CH
# m128 packed columns (f32): bgexT | gamma | beta | bgexT-bf16(bitcast)
# | dose/time on rows 64/65
BG0, GM0, BT0 = 0, NGT * BL, NGT * BL + NGT
BGBF0 = NGT * BL + 2 * NGT
DT0 = BGBF0 + NGT * BL // 2
M128_W = DT0 + BL

_DMA_ZERO_WAIT = ("InstDMACopy", "InstDMATransposeAnt", "InstTriggeredCopy")


def _split_excess_waits(nc):
    """walrus accepts at most 1 inline sync-wait per instruction (0 for
    DMA).  Move excess waits onto same-engine nops inserted before."""

    def make_nop(engine):
        bi = nc.engines[engine].nop(nofuse=True)
        ins = bi.ins
        lst = nc.cur_bb.bb.instructions
        assert lst[-1] is ins
        lst.pop()
        return ins

    for bb in nc.main_func.blocks:
        lst = bb.instructions
        i = 0
        while i < len(lst):
            ins = lst[i]
            si = getattr(ins, "sync_info", None)
            waits = list(si.on_wait) if (si and si.on_wait) else []
            limit = 0 if type(ins).__name__ in _DMA_ZERO_WAIT else 1
            if len(waits) > limit:
                keep = waits[len(waits) - limit:] if limit else []
                excess = waits[: len(waits) - limit]
                si.on_wait = keep
                pos = i
                for w in excess:
                    nop = make_nop(ins.engine)
                    nop.sync_info = mybir.SyncInfo(on_wait=[w], on_update=[])
                    lst.insert(pos, nop)
                    pos += 1
                    i += 1
            i += 1


def build_nc():
    nc = bass.Bass()

    # ---- kernel I/O (host-packed per-core layouts) ----
    in50 = nc.dram_tensor("in50", [NA, IN50_W], BF16, kind="ExternalInput")
    m128f = nc.dram_tensor("m128f", [128, M128_W], F32, kind="ExternalInput")
    w2p = nc.dram_tensor("w2p", [CH + 2, G], BF16, kind="ExternalInput")
    wgc = nc.dram_tensor("wgc", [128, 2 * NGT * 128], BF16, kind="ExternalInput")
    wgcT_d = nc.dram_tensor("wgcT_d", [128, 2 * NGT * 128], BF16, kind="ExternalInput")
    ppi8 = nc.dram_tensor("ppi8", [128, NGT * G], FP8, kind="ExternalInput")
    wffp = nc.dram_tensor("wffp", [128, NGT * GP], BF16, kind="ExternalInput")

    out_predT = nc.dram_tensor("out_predT", [128, NGT * BL], F32, kind="ExternalOutput")
    out_compT = nc.dram_tensor("out_compT", [128, NGT * BL], F32, kind="ExternalOutput")

    with tile.TileContext(nc) as tc:
        with (
            tc.tile_pool(name="const", bufs=1) as const,
            tc.tile_pool(name="sb", bufs=1) as sb,
            tc.tile_pool(name="work", bufs=4) as work,
            tc.tile_pool(name="pacc", bufs=1, space="PSUM") as pacc,
            tc.tile_pool(name="pcyc", bufs=6, space="PSUM") as pcyc,
        ):
            ones_c_bf = const.tile([128, 1], BF16)
            nc.vector.memset(ones_c_bf[:], 1.0)
            ones_c_f = const.tile([128, 1], F32)
            nc.vector.memset(ones_c_f[:], 1.0)
            ones_r_f = const.tile([1, 128], F32)
            nc.vector.memset(ones_r_f[:], 1.0)
            eps_t = const.tile([1, 1], F32)
            nc.vector.memset(eps_t[:], LN_EPS)

            _cyc_n = [0]

            def cyc(shape, dt=F32):
                _cyc_n[0] += 1
                return pcyc.tile(shape, dt, tag="cyc", name=f"cyc{_cyc_n[0]}")

            # persistent PSUM banks (one pending accumulation group per
            # bank at any time):
            #   u bank: u accumulation, then LN sum(x)
            #   oa bank: FFN pass a + LN sum(x^2)
            #   ob bank: FFN pass b
            #   oc bank: ppi row-sum passes (early), then FFN pass c
            u_ps2 = pacc.tile([H, 3 * BL], F32, tag="u")
            u_ps = u_ps2[:, 0:BL]
            stats_xx2 = u_ps2[:1, BL:3 * BL]
            stats_x = u_ps2[:1, BL:2 * BL]
            stats_x2 = u_ps2[:1, 2 * BL:3 * BL]
            # big bank: ppi row-sum partials (early), LN sum(x^2) (mid),
            # FFN output (late) -- PE is in-order, so the accumulation
            # groups in this bank open and close strictly sequentially.
            big_ps = pacc.tile([128, NGT * BL + 2 * NGT], F32, tag="big")
            o_ps = big_ps[:, 0:NGT * BL].rearrange("p (t b) -> p t b", b=BL)
            prs_ab = big_ps[:, NGT * BL:NGT * BL + 2 * NGT].rearrange(
                "p (t x) -> p t x", x=2)

            # ---- load the exp ACT table immediately (1.3us, overlaps DMA)
            warm = const.tile([1, 2], F32)
            nc.scalar.activation(warm[:, 0:1], eps_t[:], AF.Exp)

            # ================= DMA loads =================
            # SP: in50, wgc, w2, wff kt0-2, outputs
            # ACT (after its table work): wff kt6-7
            # Pool: ppi halves, m128, wff kt3-5
            in50_sb = sb.tile([NA, IN50_W], BF16)
            nc.scalar.dma_start(out=in50_sb[:], in_=in50[:, :])
            w2_sb = sb.tile([CH + 2, G], BF16)
            nc.sync.dma_start(out=w2_sb[:], in_=w2p[:, :])
            wgc_sb = sb.tile([128, 2 * NGT, 128], BF16)
            nc.sync.dma_start(out=wgc_sb[:].rearrange("p t h -> p (t h)"),
                              in_=wgc[:, :])
            wgcT = sb.tile([128, 2 * NGT, 128], BF16)
            nc.sync.dma_start(out=wgcT[:].rearrange("p t h -> p (t h)"),
                              in_=wgcT_d[:, :])
            wff_sb = sb.tile([128, NGT, GP], BF16)
            nc.sync.dma_start(out=wff_sb[:, 0:4, :].rearrange("p t k -> p (t k)"),
                              in_=wffp[:, 0:4 * GP])
            ppi_sb = sb.tile([128, NGT, G], FP8)
            nc.gpsimd.dma_start(out=ppi_sb[:, 0:4, :].rearrange("p t k -> p (t k)"),
                                in_=ppi8[:, 0:4 * G])
            nc.gpsimd.dma_start(out=ppi_sb[:, 4:8, :].rearrange("p t k -> p (t k)"),
                                in_=ppi8[:, 4 * G:])
            m128 = sb.tile([128, M128_W], F32)
            nc.gpsimd.dma_start(out=m128[:], in_=m128f[:, :])
            nc.gpsimd.dma_start(out=wff_sb[:, 4:8, :].rearrange("p t k -> p (t k)"),
                                in_=wffp[:, 4 * GP:])

            bgT = m128[:, BG0:BG0 + NGT * BL].rearrange("p (t b) -> p t b", b=BL)
            bg_bf = m128[:, BGBF0:BGBF0 + NGT * BL // 2].bitcast(BF16).rearrange(
                "p (t b) -> p t b", b=BL)

            # ================= CCE =================
            # h per sample, m-major: h_b = relu(nfT_b.T @ W1)   [50(n), 64]
            h_ps = cyc([NA, BL, CH])
            for b in range(BL):
                nc.tensor.matmul(h_ps[:, b, :],
                                 in50_sb[:, NF0 + b * NA:NF0 + (b + 1) * NA],
                                 in50_sb[:, W10:W10 + CH], start=True, stop=True)
            h_bf = sb.tile([NA, BL, CH], BF16)
            nc.vector.tensor_scalar_max(
                h_bf[:].rearrange("n b d -> n (b d)"),
                h_ps[:].rearrange("n b d -> n (b d)"), 0.0)

            wmsg = sb.tile([NA, BL * NA], BF16)
            nc.scalar.activation(wmsg[:], in50_sb[:, DIST0:DIST0 + BL * NA],
                                 AF.Exp, scale=-1.0)
            # switch ACT to the sqrt table set (Copy/Relu live there too);
            # reading wmsg forces this AFTER the Exp (no table thrash)
            nc.scalar.activation(warm[:, 1:2], wmsg[0:1, 0:1], AF.Sqrt)
            nc.vector.tensor_mul(wmsg[:], wmsg[:], in50_sb[:, ADJ0:ADJ0 + BL * NA])

            # g column-major: g_b = wmsg_b.T @ mask_b   [50(m), 1]
            g_ps = cyc([NA, BL])
            for b in range(BL):
                nc.tensor.matmul(g_ps[:, b:b + 1],
                                 wmsg[:, b * NA:(b + 1) * NA],
                                 in50_sb[:, MSK0 + b:MSK0 + b + 1],
                                 start=True, stop=True)
            g_bf = sb.tile([NA, BL], BF16)
            nc.vector.tensor_copy(g_bf[:], g_ps[:])

            # pooled column per sample: h_b.T @ g_b   [64, 1]
            pool_ps = cyc([CH, BL])
            for b in range(BL):
                nc.tensor.matmul(pool_ps[:, b:b + 1], h_bf[:, b, :],
                                 g_bf[:, b:b + 1], start=True, stop=True)

            ms_ps = cyc([1, BL])
            nc.tensor.matmul(ms_ps[:], ones_c_bf[:NA, :],
                             in50_sb[:, MSK0:MSK0 + BL], start=True, stop=True)
            ms_sb = sb.tile([1, BL], F32)
            nc.vector.tensor_scalar_max(ms_sb[:], ms_ps[:], 1.0)
            rms = sb.tile([1, BL], F32)
            nc.vector.reciprocal(rms[:], ms_sb[:])
            rb_ps = cyc([CH, BL])
            nc.tensor.matmul(rb_ps[:], ones_r_f[:1, :CH], rms[:], start=True, stop=True)
            rb_sb = sb.tile([CH, BL], F32)
            nc.vector.tensor_copy(rb_sb[:], rb_ps[:])
            pooledT_bf = sb.tile([CH + 2, BL], BF16)
            nc.vector.tensor_mul(pooledT_bf[:CH, :], pool_ps[:], rb_sb[:])
            nc.vector.tensor_copy(pooledT_bf[64:66, :], m128[64:66, DT0:DT0 + BL])

            # comp gene-major (dose/time folded into W2's last 2 rows)
            compT = sb.tile([128, NGT, BL], F32)
            compT_bf = sb.tile([128, NGT, BL], BF16)
            nc.vector.memset(compT[:, 7, :], 0.0)
            nc.vector.memset(compT_bf[:, 7, :], 0.0)
            for half in range(2):
                cT_ps = cyc([128, 4, BL])
                for j in range(4):
                    gt = half * 4 + j
                    gs, gn = gt * 128, (82 if gt == 7 else 128)
                    nc.tensor.matmul(cT_ps[:gn, j, :], w2_sb[:, gs:gs + gn],
                                     pooledT_bf[:], start=True, stop=True)
                if half == 0:
                    nc.scalar.copy(compT[:, 0:4, :], cT_ps[:])
                    nc.vector.tensor_copy(compT_bf[:, 0:4, :], cT_ps[:])
                else:
                    nc.scalar.copy(compT[:, 4:7, :], cT_ps[:, 0:3, :])
                    nc.scalar.copy(compT[:82, 7, :], cT_ps[:82, 3, :])
                    nc.vector.tensor_copy(compT_bf[:, 4:7, :], cT_ps[:, 0:3, :])
                    nc.vector.tensor_copy(compT_bf[:82, 7, :], cT_ps[:82, 3, :])
            nc.sync.dma_start(out=out_compT[:, :],
                              in_=compT[:].rearrange("p t b -> p (t b)"))

            # ================= u = b_gex@wg + comp@wc  [H, BL] ==========
            for t in range(NGT):
                nc.tensor.matmul(u_ps, wgc_sb[:, t, :], bg_bf[:, t, :],
                                 start=(t == 0), stop=False)
            for t in range(NGT):
                nc.tensor.matmul(u_ps, wgc_sb[:, NGT + t, :], compT_bf[:, t, :],
                                 start=False, stop=(t == NGT - 1))
            u_bf = sb.tile([H, BL], BF16)
            nc.vector.tensor_scalar_mul(u_bf[:], u_ps, ISH)  # fold 1/sqrt(H)


            # ============ ppi row sums: stationary k-major blocks ========
            # prs_ab[:, t, kc] += ones over each arrived ppi chunk
            prs = sb.tile([128, NGT], F32)
            nc.vector.memset(prs[:], 0.0)
            for kc in range(2):
                for t in range(NGT):
                    tn = 82 if t == 7 else 128
                    for kt in range(4 * kc, 4 * kc + 4):
                        nc.tensor.matmul(
                            prs_ab[:tn, t, kc:kc + 1],
                            ppi_sb[:, kt, t * 128:t * 128 + tn],
                            ones_c_bf[:, :1],
                            start=(kt == 4 * kc), stop=(kt == 4 * kc + 3))
            prs_a_sb = sb.tile([128, NGT], F32)
            nc.vector.tensor_copy(prs_a_sb[:, 0:7], prs_ab[:, 0:7, 0])
            nc.vector.tensor_copy(prs_a_sb[:82, 7:8], prs_ab[:82, 7, 0:1])
            nc.vector.tensor_add(prs[:, 0:7],
                                 prs_a_sb[:, 0:7], prs_ab[:, 0:7, 1])
            nc.vector.tensor_add(prs[:82, 7:8],
                                 prs_a_sb[:82, 7:8], prs_ab[:82, 7, 1:2])

            # ========== A/C, pred, LN stats (gene-major, 4-tile batches) =
            psq = sb.tile([128, NGT, 2, BL], F32)
            predT = psq[:, :, 0, :]
            sq = psq[:, :, 1, :]
            for half in range(2):
                hs = slice(4 * half, 4 * half + 4)
                AC_ps = cyc([128, 4, 2, BL])
                for j in range(4):
                    t = 4 * half + j
                    nc.tensor.matmul(AC_ps[:, j, 0, :], wgcT[:, t, :], u_bf[:],
                                     start=True, stop=True)
                    nc.tensor.matmul(AC_ps[:, j, 1, :], wgcT[:, NGT + t, :],
                                     u_bf[:], start=True, stop=True)
                t1 = work.tile([128, 4, BL], F32, tag="t1")
                nc.vector.tensor_mul(t1[:], bgT[:, hs, :], AC_ps[:, :, 0, :])
                t2 = work.tile([128, 4, BL], F32, tag="t2")
                nc.vector.tensor_mul(t2[:], compT[:, hs, :], AC_ps[:, :, 1, :])
                nc.gpsimd.tensor_add(t1[:], t1[:], t2[:])
                for j in range(4):
                    t = 4 * half + j
                    nc.gpsimd.tensor_scalar(t1[:, j, :], t1[:, j, :],
                                            prs[:, t:t + 1], 0.0,
                                            op0=OP.add, op1=OP.add)
                nc.vector.tensor_mul(predT[:, hs, :], t1[:], bgT[:, hs, :])
                nc.gpsimd.tensor_mul(sq[:, hs, :], predT[:, hs, :], predT[:, hs, :])
                for j in range(4):
                    t = 4 * half + j
                    nc.tensor.matmul(stats_xx2, ones_c_f[:],
                                     psq[:, t, :, :].rearrange("p x b -> p (x b)"),
                                     start=(t == 0), stop=(t == NGT - 1))

            # ================= LayerNorm + ReLU =================
            mex = sb.tile([1, 2 * BL], F32)      # [mu || ex2]
            nc.vector.tensor_scalar_mul(mex[:], stats_xx2, 1.0 / G)
            mu = mex[:, 0:BL]
            mu2 = sb.tile([1, BL], F32)
            nc.vector.tensor_mul(mu2[:], mu, mu)
            var = sb.tile([1, BL], F32)
            nc.vector.tensor_sub(var[:], mex[:, BL:2 * BL], mu2[:])
            sd = sb.tile([1, BL], F32)
            nc.scalar.activation(sd[:], var[:], AF.Sqrt, bias=eps_t[:1, 0:1])
            rstd = sb.tile([1, BL], F32)
            nc.vector.reciprocal(rstd[:], sd[:])
            # broadcast mu/rstd replicated over the 4-tile batch dim
            rep = sb.tile([1, 2, 4, BL], F32)
            nc.vector.tensor_copy(
                rep[:, 0, :, :],
                mu.rearrange("o (x b) -> o x b", x=1).broadcast_to([1, 4, BL]))
            nc.gpsimd.tensor_copy(
                rep[:, 1, :, :],
                rstd[:].rearrange("o (x b) -> o x b", x=1).broadcast_to([1, 4, BL]))
            rep_ps = cyc([128, 2, 4, BL])
            nc.tensor.matmul(rep_ps[:].rearrange("p x r b -> p (x r b)"),
                             ones_r_f[:],
                             rep[:].rearrange("o x r b -> o (x r b)"),
                             start=True, stop=True)

            xn = sb.tile([128, NGT, BL], BF16)
            for half in range(2):
                hs = slice(4 * half, 4 * half + 4)
                xm = work.tile([128, 4, BL], F32, tag="xm")
                nc.vector.tensor_sub(xm[:], predT[:, hs, :], rep_ps[:, 0])
                nc.vector.tensor_mul(xm[:], xm[:], rep_ps[:, 1])
                for j in range(4):
                    t = 4 * half + j
                    nc.gpsimd.tensor_scalar(xm[:, j, :], xm[:, j, :],
                                            m128[:, GM0 + t:GM0 + t + 1],
                                            m128[:, BT0 + t:BT0 + t + 1],
                                            op0=OP.mult, op1=OP.add)
                    nc.gpsimd.tensor_scalar_max(xn[:, t, :], xm[:, j, :], 0.0)

            # ================= FFN: out[g,b] = sum_k W_ff[k,g] xn[k,b] ==
            for mt in range(NGT):
                for kt in range(NGT):
                    nc.tensor.matmul(
                        o_ps[:, mt, :],
                        wff_sb[:, kt, mt * 128:(mt + 1) * 128],
                        xn[:, kt, :],
                        start=(kt == 0), stop=(kt == NGT - 1))
            o_sb = sb.tile([128, NGT, BL], F32)
            nc.vector.tensor_copy(o_sb[:], o_ps)
            nc.sync.dma_start(out=out_predT[:, :],
                              in_=o_sb[:].rearrange("p t b -> p (t b)"))

    _split_excess_waits(nc)
    return nc


# ================= host-side packing / unpacking =================

def _tilepack(mat, width):
    """[rows<=1024, width] -> [128, NGT*width]: row r lands at
    (partition r%128, block r//128), zero-padded to 1024 rows."""
    out = np.zeros((GP, width), np.float32)
    out[:mat.shape[0]] = mat
    return np.ascontiguousarray(
        out.reshape(NGT, 128, width).transpose(1, 0, 2).reshape(128, NGT * width))


def make_in_maps(inputs):
    inp = {k: np.asarray(v, dtype=np.float32) for k, v in inputs.items()}

    wg_p = _tilepack(inp["w_gex"], H)
    wc_p = _tilepack(inp["w_comp"], H)
    wgc = np.concatenate([wg_p, wc_p], axis=1).astype(NP_BF16)

    def _tpackT(mat):                       # [G, H] -> [128(h), NGT*128]
        out = np.zeros((GP, H), np.float32)
        out[:mat.shape[0]] = mat
        return out.reshape(NGT, 128, H).transpose(2, 0, 1).reshape(H, NGT * 128)

    wgcT_d = np.ascontiguousarray(np.concatenate(
        [_tpackT(inp["w_gex"]), _tpackT(inp["w_comp"])], axis=1)).astype(NP_BF16)
    # ppi packed k-major (transposed) so each 128x128 block is a
    # stationary lhsT for the row-sum matmuls
    ppi8 = _tilepack(np.ascontiguousarray(inp["ppi_adj"].T), G).astype(NP_FP8)
    wffp = _tilepack(np.pad(inp["W_ff"], ((0, 0), (0, GP - G))), GP).astype(NP_BF16)
    w2p = np.concatenate([inp["W2"], inp["w_dose"], inp["w_time"]],
                         axis=0).astype(NP_BF16)

    gz = np.zeros(GP, np.float32)
    gz[:G] = inp["ln_gamma"]
    gcols = gz.reshape(NGT, 128).T
    bz = np.zeros(GP, np.float32)
    bz[:G] = inp["ln_beta"]
    bcols = bz.reshape(NGT, 128).T

    W1p = np.zeros((NA, CH), np.float32)
    W1p[:FEAT] = inp["W1"]

    in_maps = []
    for c in range(NCORES):
        s = slice(c * BL, (c + 1) * BL)
        adjT = inp["adj_matrix"][s].transpose(1, 0, 2).reshape(NA, BL * NA)
        distT = inp["dist_matrix"][s].transpose(1, 0, 2).reshape(NA, BL * NA)
        maskT = inp["mask"][s].T
        nfT = np.zeros((NA, BL * NA), np.float32)
        nfT[:FEAT] = inp["node_feat"][s].transpose(2, 0, 1).reshape(FEAT, BL * NA)
        in50 = np.concatenate([adjT, distT, maskT, nfT, W1p], axis=1).astype(NP_BF16)

        bgT = _tilepack(inp["b_gex"][s].T, BL)          # [128, NGT*BL] f32
        bgbf_bits = np.ascontiguousarray(
            bgT.astype(NP_BF16)).view(np.float32)       # 64 bf16 -> 32 f32
        dt_cols = np.zeros((128, BL), np.float32)
        dt_cols[64] = inp["dose"][s, 0]
        dt_cols[65] = inp["time"][s, 0]
        m128f = np.ascontiguousarray(np.concatenate(
            [bgT, gcols, bcols, bgbf_bits, dt_cols], axis=1).astype(np.float32))

        in_maps.append({
            "in50": np.ascontiguousarray(in50),
            "m128f": m128f,
            "w2p": np.ascontiguousarray(w2p),
            "wgc": np.ascontiguousarray(wgc),
            "wgcT_d": np.ascontiguousarray(wgcT_d),
            "ppi8": np.ascontiguousarray(ppi8),
            "wffp": np.ascontiguousarray(wffp),
        })
    return in_maps


def _unpack_out(arr):
    """[128, NGT*BL] gene-major -> [BL, G] batch-major."""
    return np.ascontiguousarray(
        np.asarray(arr, dtype=np.float32).reshape(128, NGT, BL)
        .transpose(2, 1, 0).reshape(BL, GP)[:, :G])


def kernel(**inputs):
    nc = build_nc()
    in_maps = make_in_maps(inputs)
    r = run_bass_kernel_spmd(nc, in_maps, list(range(NCORES)))
    pred = np.concatenate(
        [_unpack_out(r.results[c]["out_predT"]) for c in range(NCORES)], axis=0)
    comp = np.concatenate(
        [_unpack_out(r.results[c]["out_compT"]) for c in range(NCORES)], axis=0)
    return pred, comp
